# revision 14
# baseline (speedup 1.0000x reference)
"""Trainium2 Bass kernel for LogSpaceMinGRU.

Math: the reference computes, per (batch, channel), a log-space Heinsen scan:
    hg = x @ W.T ; hidden, gate = split(hg)
    log_h = cumulative-logsumexp formulation of  h_t = (1-z_t) h_{t-1} + z_t g(hidden_t)
    out = exp(log_h)
with z = sigmoid(gate), g(x) = relu(x)+0.5 (x>=0) | sigmoid(x) (x<0).

The log-space form exists only for numerical stability.  In linear space the
recurrence h_t = c_t*h_{t-1} + v_t (c = sigmoid(-gate) in (0,1),
v = z*g >= 0) is a convex-combination update, perfectly stable in f32, and
maps 1:1 onto the TRN2 DVE `tensor_tensor_scan` instruction
(state = data0*state + data1 along the free dim).  Verified numerically:
the linear-space f32 result is ~7e-7 from the f64 ground truth, while the
f32 log-space reference itself carries ~2e-4 of rounding error.

Note g(x) = max(sigmoid(x), x + 0.5) for all x (equality at 0; sigmoid is
above the line for x<0, below for x>=0), which gives a branch-free form.

Sharding over 8 cores: batch (4) x output-feature-half (2).  Each core
computes, for one batch b and one 512-wide feature slice:
    hg_slice = x[b] @ W_slice.T  -> [4096, 1024] (512 hidden | 512 gate)
    h = scan(...)                -> [512, 4096] (channel-major)
The host pre-transposes x[b] to [d, s] (free) and post-transposes the
channel-major output back, so the device never pays for transposes.

Schedule (v2): the kernel is PE-bound — 512 matmuls x 512 moving cols =
262144 PE cycles = 109.2 us/core at 2.4 GHz is the fp16 dense floor
(fp8/DoubleRow would halve it but e4m3 quantization of x and W puts
3.7e-2 of error on the output — measured — vs the 2e-2 gate).  What this
schedule optimizes is everything around that floor:
  - Seq chunk Q=512: each PSUM chunk is exactly one bank (8 tags, bufs=1
    = all 8 banks), consumer passes are half as long as with Q=1024.
  - Startup: per-k DMAs interleaved (x_k, w_k-hidden, w_k-gate pieces of
    128 KiB each) and the first octant's matmuls run k-OUTER across 6
    PSUM chunks, so the PE starts after ~2 transfers instead of all 16
    (sim: 6.3 us -> 2.7 us of PE idle at start).
  - Steady state: 8 per-k x DMAs per octant spread across HW queues (a
    merged 3D-AP DMA sims the same but measured ~16 us/rep slower on HW).
  - Tail: the last (p, octant) is consumed in two 256-col chunks with the
    output DMAs issued from ACT/SP alternately, shortening the exposed
    consumer+DMA chain after the final matmul.
  - Output h is written fp16 (host upcasts): halves output-DMA bytes; adds
    only ~2e-4 of scale-relative rounding (measured 4.5e-4 total on HW vs
    the 2e-2 gate), HW steady state neutral-to-better (paired bench).
Cost-model sim: 126.8 us (v1) -> 116.9 us; HW steady-state (differential
bench over in-NEFF reps) is at the PE stream floor.
"""

import sys

sys.path.insert(0, "/opt/trn_rl_repo")

import numpy as np

_B, _S, _D = 4, 4096, 1024
_CH = 512          # channels per core (feature slice)
_Q = 512           # sequence octant
_NQ = _S // _Q     # 8
_NK = _D // 128    # 8 contraction tiles
_NP = _CH // 128   # 4 channel tiles

_programs = {}


def _build_program(reps=1, tailsplit=True):
    import concourse.bass as bass  # noqa: F401  (registers engine classes)
    import concourse.tile as tile
    from concourse import bacc, mybir

    f32 = mybir.dt.float32
    f16 = mybir.dt.float16
    AF = mybir.ActivationFunctionType
    OP = mybir.AluOpType

    nc = bacc.Bacc("TRN2", target_bir_lowering=False, debug=False)
    x_d = nc.dram_tensor("x", [_D, _S], f16, kind="ExternalInput").ap()
    w_d = nc.dram_tensor("w", [_D, 2 * _CH], f16, kind="ExternalInput").ap()
    h_d = nc.dram_tensor("h", [_CH, _S], f16, kind="ExternalOutput").ap()

    with tile.TileContext(nc) as tc:
        with (
            tc.tile_pool(name="wp", bufs=1) as wp,
            tc.tile_pool(name="xp", bufs=3) as xp,
            tc.tile_pool(name="ps", bufs=1, space="PSUM") as ps,
            tc.tile_pool(name="sb", bufs=2) as sb,
            tc.tile_pool(name="hp", bufs=2) as hp,
        ):
            # Resident weights [d, e_local]: 8 k-tiles of [128, 1024].
            # First octant's x tiles interleaved with the w pieces so the
            # first k-chains unblock early (every piece is 128 KiB).
            wt = []
            x0 = []
            for k in range(_NK):
                xt = xp.tile([128, _Q], f16, tag=f"x{k}")
                nc.sync.dma_start(xt[:], x_d[k * 128 : (k + 1) * 128, 0:_Q])
                x0.append(xt)
                t = wp.tile([128, 2 * _CH], f16, tag=f"w{k}")
                nc.sync.dma_start(t[:, 0:_CH], w_d[k * 128 : (k + 1) * 128, 0:_CH])
                nc.sync.dma_start(
                    t[:, _CH : 2 * _CH], w_d[k * 128 : (k + 1) * 128, _CH : 2 * _CH]
                )
                wt.append(t)

            def consume(p, q, psh, psg, hprev, sq):
                """ACT/DVE/Pool pipeline + scan + output DMA for one
                (p, octant) PSUM pair; returns the h tile."""
                sh = sb.tile([128, _Q], f32, tag=f"sh{p}")
                nc.scalar.activation(sh[:], psh[:], AF.Sigmoid)
                g = sb.tile([128, _Q], f32, tag=f"g{p}")
                nc.vector.scalar_tensor_tensor(
                    g[:], psh[:], 0.5, sh[:], OP.add, OP.max
                )
                cc = sb.tile([128, _Q], f32, tag=f"c{p}")
                nc.scalar.activation(cc[:], psg[:], AF.Sigmoid, scale=-1.0)
                z = sb.tile([128, _Q], f32, tag=f"z{p}")
                nc.gpsimd.tensor_scalar(z[:], cc[:], -1.0, 1.0, OP.mult, OP.add)
                v = sb.tile([128, _Q], f32, tag=f"v{p}")
                nc.gpsimd.tensor_mul(v[:], z[:], g[:])
                h = hp.tile([128, _Q], f16, tag=f"h{p}")
                init = 0.0 if (q == 0) else hprev[:, _Q - 1 : _Q]
                nc.vector.tensor_tensor_scan(
                    h[:], cc[:], v[:], init, OP.mult, OP.add
                )
                nc.sync.dma_start(h_d[p * 128 : (p + 1) * 128, sq], h[:])
                return h

            def consume_split(p, q, psh, psg, hprev, sq, n=2):
                """Tail variant: process the octant in n column chunks so
                the final consumer chain and DMA are 1/n the length.  The
                hidden-path ops (sh, g) precompute during the gate matmul
                chain; the post-matmul critical path is cc -> z -> v -> scan
                -> DMA, with the two chunk DMAs issued from ACT and SP."""
                C = _Q // n
                sh = sb.tile([128, _Q], f32, tag=f"sh{p}")
                g = sb.tile([128, _Q], f32, tag=f"g{p}")
                cc = sb.tile([128, _Q], f32, tag=f"c{p}")
                z = sb.tile([128, _Q], f32, tag=f"z{p}")
                v = sb.tile([128, _Q], f32, tag=f"v{p}")
                h = hp.tile([128, _Q], f16, tag=f"h{p}")
                for i in range(n):
                    cs = slice(i * C, (i + 1) * C)
                    nc.scalar.activation(sh[:, cs], psh[:, cs], AF.Sigmoid)
                    nc.vector.scalar_tensor_tensor(
                        g[:, cs], psh[:, cs], 0.5, sh[:, cs], OP.add, OP.max
                    )
                    nc.scalar.activation(cc[:, cs], psg[:, cs], AF.Sigmoid,
                                         scale=-1.0)
                    nc.gpsimd.tensor_scalar(z[:, cs], cc[:, cs], -1.0, 1.0,
                                            OP.mult, OP.add)
                    nc.gpsimd.tensor_mul(v[:, cs], z[:, cs], g[:, cs])
                    if i == 0:
                        init = 0.0 if (q == 0) else hprev[:, _Q - 1 : _Q]
                    else:
                        init = h[:, i * C - 1 : i * C]
                    nc.vector.tensor_tensor_scan(
                        h[:, cs], cc[:, cs], v[:, cs], init, OP.mult, OP.add
                    )
                    eng = nc.scalar if i % 2 == 0 else nc.sync
                    eng.dma_start(
                        h_d[p * 128 : (p + 1) * 128,
                            sq.start + i * C : sq.start + (i + 1) * C],
                        h[:, cs],
                    )
                return h

            hprev = [None] * _NP
            for qq in range(_NQ * reps):
                q = qq % _NQ
                sq = slice(q * _Q, (q + 1) * _Q)
                if q == 0:
                    xq = x0
                else:
                    # per-k DMAs: 8 x 128 KiB spread across HW queues.  (A
                    # single merged 1 MiB 3D-AP DMA sims the same but
                    # measured ~16 us/rep slower on HW — one dma_start
                    # doesn't fan out across enough queues.)
                    xq = []
                    for k in range(_NK):
                        t = xp.tile([128, _Q], f16, tag=f"x{k}")
                        nc.sync.dma_start(t[:], x_d[k * 128 : (k + 1) * 128, sq])
                        xq.append(t)

                if q == 0:
                    # warm-up: k-outer over p0-2 (6 PSUM banks) so the PE
                    # starts on k=0 as soon as its first DMA pieces land
                    psh = [ps.tile([128, _Q], f32, tag=f"ph{p}", name=f"psh{p}")
                           for p in range(_NP)]
                    psg = [ps.tile([128, _Q], f32, tag=f"pg{p}", name=f"psg{p}")
                           for p in range(_NP)]
                    for k in range(_NK):
                        for p in range(3):
                            nc.tensor.matmul(
                                psh[p][:], wt[k][:, p * 128 : (p + 1) * 128],
                                xq[k][:], start=(k == 0), stop=(k == _NK - 1),
                                skip_group_check=True,
                            )
                        for p in range(3):
                            ec = _NP + p
                            nc.tensor.matmul(
                                psg[p][:], wt[k][:, ec * 128 : (ec + 1) * 128],
                                xq[k][:], start=(k == 0), stop=(k == _NK - 1),
                                skip_group_check=True,
                            )
                    # p3 k-inner right after (x resident by then); its
                    # matmuls cover the p0-2 consumer drain
                    for ec, pst in ((3, psh[3]), (_NP + 3, psg[3])):
                        for k in range(_NK):
                            nc.tensor.matmul(
                                pst[:], wt[k][:, ec * 128 : (ec + 1) * 128],
                                xq[k][:], start=(k == 0), stop=(k == _NK - 1),
                            )
                    for p in range(_NP):
                        hprev[p] = consume(p, q, psh[p], psg[p], hprev[p], sq)
                    continue

                for p in range(_NP):
                    psh = ps.tile([128, _Q], f32, tag=f"ph{p}")
                    psg = ps.tile([128, _Q], f32, tag=f"pg{p}")
                    last = (qq == _NQ * reps - 1) and (p == _NP - 1)
                    if last and tailsplit:
                        for ec, pst in ((p, psh), (_NP + p, psg)):
                            for k in range(_NK):
                                nc.tensor.matmul(
                                    pst[:], wt[k][:, ec * 128 : (ec + 1) * 128],
                                    xq[k][:], start=(k == 0), stop=(k == _NK - 1),
                                )
                        hprev[p] = consume_split(p, q, psh, psg, hprev[p], sq)
                        continue
                    for ec, pst in ((p, psh), (_NP + p, psg)):
                        for k in range(_NK):
                            nc.tensor.matmul(
                                pst[:], wt[k][:, ec * 128 : (ec + 1) * 128],
                                xq[k][:], start=(k == 0), stop=(k == _NK - 1),
                            )
                    hprev[p] = consume(p, q, psh, psg, hprev[p], sq)

    nc.compile()
    return nc


def _get_program(reps=1, mm16=True, mm_only=False, **cfg):
    key = (reps, tuple(sorted(cfg.items())))
    if key not in _programs:
        _programs[key] = _build_program(reps, **cfg)
    return _programs[key]


def _shard_inputs(x, W, mm16=True):
    x = np.ascontiguousarray(x, dtype=np.float32)
    W = np.ascontiguousarray(W, dtype=np.float32)
    in_maps = []
    xT = [np.ascontiguousarray(x[b].T.astype(np.float16)) for b in range(_B)]
    for core in range(_B * 2):
        b, f = divmod(core, 2)
        w_slice = np.concatenate(
            [W[f * _CH : (f + 1) * _CH], W[_D + f * _CH : _D + (f + 1) * _CH]],
            axis=0,
        )  # [1024 (e_local), 1024 (d)]
        wT = np.ascontiguousarray(w_slice.T.astype(np.float16))  # [d, e_local]
        in_maps.append({"x": xT[b], "w": wT})
    return in_maps


def _unshard(results):
    out = np.empty((_B, _S, _D), dtype=np.float32)
    for core in range(_B * 2):
        b, f = divmod(core, 2)
        out[b, :, f * _CH : (f + 1) * _CH] = results[core]["h"].T
    return out


def run_sharded(x, W, reps=1, mm16=True, mm_only=False, **kwargs):
    """Run the SPMD kernel; returns (output, BassKernelResults)."""
    from concourse.bass_utils import run_bass_kernel_spmd

    cfg = {k: kwargs.pop(k) for k in list(kwargs) if k in ("tailsplit",)}
    run_kwargs = kwargs
    nc = _get_program(reps, **cfg)
    in_maps = _shard_inputs(x, W)
    last_err = None
    for attempt in range(3):
        try:
            res = run_bass_kernel_spmd(nc, in_maps, list(range(_B * 2)), **run_kwargs)
            return _unshard(res.results), res
        except Exception as e:  # transient device errors (NRT_EXEC_UNIT_...)
            last_err = e
    raise last_err


def kernel(x, W):
    out, _ = run_sharded(x, W)
    return out
